# revision 31
# baseline (speedup 1.0000x reference)
"""Causal self-attention Trainium2 kernel (8-core SPMD, tensor-parallel over heads).

Reference computation (B=4, T=2048, C=1024, NH=16, HS=64):
    qkv = x @ w_attn + b_attn ; split q,k,v ; per-head causal softmax(q k^T / sqrt(HS)) @ v
    y = concat_heads @ w_proj + b_proj

Sharding: each of the 8 cores owns 2 heads (128 of the 1024 channels).
Per core:  qkv projection for its head-slice (x^T replicated), full causal
attention for its 2 heads x 4 batches, and a partial output projection
(w_proj row-slice).  Host sums the 8 partial projections and adds b_proj.

All matmul operands are bf16 (psum accumulation stays fp32): 1 cycle/row at
any moving-dim size, half the SBUF traffic, and x/w/y DMA bytes halve.
The two heads' score matmuls contract over only 64 partitions each, so they
are row-tiled onto PE tile positions (0,*) and (64,*) and run concurrently.
PV matmuls for q-block qb-1 are interleaved chunk-by-chunk with the score
matmuls of q-block qb: the PV stream (no LDWEIGHTS pressure) hides the
score matmuls' LDWEIGHTS cost, and PV never waits on the exp tail of its
own q-block.  Causal masking adds -30000 to the two diagonal k-chunks in
PSUM before exp.  Softmax row-sums come from an appended ones-column in V.
"""

import numpy as np
import ml_dtypes

B, T, C, NH = 4, 2048, 1024, 16
HS = C // NH            # 64
NCORES = 8
NH_LOC = NH // NCORES   # 2 heads per core
HS2 = NH_LOC * HS       # 128
TOK = B * T             # 8192
TB = T                  # tokens per batch
SCALE = 1.0 / float(np.sqrt(HS))
NEG = -30000.0

QB = 256                # q-block (free dim of S^T / PV matmuls)
NQB = TB // QB          # 8 q-blocks per batch
KC = 128                # k-chunk
EXPG = 2                # k-chunks per exp() call ([128, 2, 512] psum)

_CACHE = {}


def _build():
    import concourse.bass as bass
    import concourse.tile as tile
    from concourse import bacc, mybir

    dt = mybir.dt
    f32, bf16 = dt.float32, dt.bfloat16

    nc = bacc.Bacc(None, target_bir_lowering=False, debug=False)
    with tile.TileContext(nc) as tc:
        with tc.tile_pool(name="dram", bufs=1, space="DRAM") as dram:
            xT = dram.tile([C, TOK], bf16, kind="ExternalInput", name="xT", uniquify=False)
            wq_d = dram.tile([C, HS2], bf16, kind="ExternalInput", name="wq", uniquify=False)
            wk_d = dram.tile([C, HS2], bf16, kind="ExternalInput", name="wk", uniquify=False)
            wv_d = dram.tile([C, HS2], bf16, kind="ExternalInput", name="wv", uniquify=False)
            wp_d = dram.tile([HS2, C], bf16, kind="ExternalInput", name="wp", uniquify=False)
            bq_d = dram.tile([HS2, 1], f32, kind="ExternalInput", name="bq", uniquify=False)
            bk_d = dram.tile([HS2, 1], f32, kind="ExternalInput", name="bk", uniquify=False)
            bv_d = dram.tile([HS2, 1], f32, kind="ExternalInput", name="bv", uniquify=False)
            mn_d = dram.tile([KC, 2 * (QB + KC)], f32, kind="ExternalInput", name="mneg", uniquify=False)
            y_d = dram.tile([TOK, C], bf16, kind="ExternalOutput", name="y", uniquify=False)

            lb_d = [dram.tile([TB], f32, name=f"lb{i}", uniquify=False) for i in range(2)]
            _emit(nc, tc, bass, mybir, locals())
    nc.compile()
    return nc


def _emit(nc, tc, bass, mybir, io):
    import concourse.tile as tile

    dt = mybir.dt
    f32, bf16 = dt.float32, dt.bfloat16
    Exp = mybir.ActivationFunctionType.Exp

    xT, wq_d, wk_d, wv_d, wp_d = io["xT"], io["wq_d"], io["wk_d"], io["wv_d"], io["wp_d"]
    bq_d, bk_d, bv_d, mn_d, y_d = (
        io["bq_d"], io["bk_d"], io["bv_d"], io["mn_d"], io["y_d"])
    lb_d = io["lb_d"]

    with (
        tc.tile_pool(name="consts", bufs=1) as consts,
        tc.tile_pool(name="xt", bufs=10) as xtp,
        tc.tile_pool(name="qt", bufs=2) as qtp,
        tc.tile_pool(name="kt", bufs=2) as ktp,
        tc.tile_pool(name="vaug", bufs=4) as vaugp,
        tc.tile_pool(name="pt", bufs=3) as ptp,
        tc.tile_pool(name="ytmp", bufs=2) as ytmpp,
        tc.tile_pool(name="lrp", bufs=4) as lrp,
        tc.tile_pool(name="recp", bufs=2) as recp,
        tc.tile_pool(name="yt", bufs=2) as ytpool,
        tc.tile_pool(name="outsb", bufs=5) as outp,
        tc.tile_pool(name="mmps", bufs=2, space="PSUM") as mmps,
        tc.tile_pool(name="stps", bufs=2, space="PSUM") as stps,
        tc.tile_pool(name="pvps", bufs=2, space="PSUM") as pvps,
    ):
        # ---- constants -------------------------------------------------
        # wq gates the very first matmul: two half-size DMAs on the SP queue
        # (ahead of the x tiles), everything else via the scalar/vector DMA
        # queues so SP reaches the F0 x loads immediately.
        wq_sb = consts.tile([128, 8, 128], bf16, name="wq_sb")
        wk_sb = consts.tile([128, 8, 128], bf16, name="wk_sb")
        wv_sb = consts.tile([128, 8, 128], bf16, name="wv_sb")
        for sb, d, eng in ((wq_sb, wq_d, nc.sync), (wk_sb, wk_d, nc.scalar),
                           (wv_sb, wv_d, nc.scalar)):
            dr = d.rearrange("(cc p) m -> p cc m", p=128)
            for hh in range(2):
                eng.dma_start(sb[:, 4 * hh:4 * hh + 4, :], dr[:, 4 * hh:4 * hh + 4, :])
        wp_sb = consts.tile([HS2, C], bf16, name="wp_sb")
        nc.gpsimd.dma_start(wp_sb[:], wp_d[:])
        bq_sb = consts.tile([HS2, 1], f32, name="bq_sb")
        bk_sb = consts.tile([HS2, 1], f32, name="bk_sb")
        bv_sb = consts.tile([HS2, 1], f32, name="bv_sb")
        for sb, d in ((bq_sb, bq_d), (bk_sb, bk_d), (bv_sb, bv_d)):
            nc.gpsimd.dma_start(sb[:], d[:])
        mn_sb = consts.tile([KC, 2, QB + KC], f32, name="mn_sb")
        nc.scalar.dma_start(mn_sb[:], mn_d[:])
        bv_bc = consts.tile([128, HS2], f32, name="bv_bc")
        nc.scalar.dma_start(bv_bc[:], bass.AP(bv_d.tensor, 0, [[0, 128], [1, HS2]]))

        # PE warm-up: ~4us of junk matmuls on a memset scratch tile while
        # the first x/weight DMAs are still in flight.  This trips the HAM
        # activity window early so the first real matmuls run at 2.4 GHz
        # instead of the cold 1.2 GHz half-clock.
        warm = consts.tile([128, 512], bf16, name="warm")
        nc.vector.memset(warm[:], 0.0)
        warm_ps = mmps.tile([128, 512], f32, name="mm", tag="mm")
        for _ in range(10):
            nc.tensor.matmul(warm_ps[:], warm[:, 0:128], warm[:],
                             start=True, stop=True)

        def gen_qkv(b, st):
            """QKV projection units for batch b: per F-block a Q unit, a K
            unit, and a V unit.  qT/kT hold both heads' 64 dims stacked on
            the partition axis; V is computed directly in [token, hs]
            layout (bf16 rhs streams at 1 cyc/row even at N=128)."""
            base = b * TB
            qT = qtp.tile([128, TB], bf16, name="qT")
            kT = ktp.tile([128, TB], bf16, name="kT")
            st["qT"] = qT
            st["kT"] = kT
            va = [vaugp.tile([128, TB // KC, HS + 1], bf16, name=f"vaug{h}")
                  for h in range(NH_LOC)]
            st["va"] = va
            for h in range(NH_LOC):
                nc.vector.memset(va[h][:, :, HS:HS + 1], 1.0)
            xTr = xT.rearrange("(g p) m -> p g m", p=128)
            for F in range(4):
                cols = bass.ds(base + F * 512, 512)
                lcols = bass.ds(F * 512, 512)
                # 4 DMAs per F-block (2 c-chunks each): keeps 4 DMA queues
                # busy without saturating the SP sequencer (~600ns per issue).
                # The first F-block of batch 0 is on the critical path, so it
                # uses 8 half-size DMAs for ~half the arrival latency.
                xts = []
                for t in range(4):
                    xt = xtp.tile([128, 2, 512], bf16, name="xt")
                    if b == 0 and F == 0:
                        for k in range(2):
                            nc.sync.dma_start(xt[:, k, :], xTr[:, 2 * t + k, cols])
                    else:
                        nc.sync.dma_start(xt[:], xTr[:, 2 * t:2 * t + 2, cols])
                    xts.append(xt)

                def xs(cc):
                    return xts[cc // 2][:, cc % 2, :]

                ps_q = mmps.tile([128, 512], f32, name="mm", tag="mm")
                for cc in range(8):
                    nc.tensor.matmul(ps_q[:], wq_sb[:, cc, :], xs(cc),
                                     start=(cc == 0), stop=(cc == 7))
                nc.vector.tensor_scalar_add(qT[:, lcols], ps_q[:], bq_sb[:])
                yield
                ps_k = mmps.tile([128, 512], f32, name="mm", tag="mm")
                for cc in range(8):
                    nc.tensor.matmul(ps_k[:], wk_sb[:, cc, :], xs(cc),
                                     start=(cc == 0), stop=(cc == 7))
                nc.vector.tensor_scalar_add(kT[:, lcols], ps_k[:], bk_sb[:])
                yield
                psv = mmps.tile([128, 512], f32, name="mm", tag="mm")
                for tj in range(4):
                    tc128 = bass.ds(tj * 128, 128)
                    for cc in range(8):
                        nc.tensor.matmul(psv[:, tc128], xs(cc)[:, tc128], wv_sb[:, cc, :],
                                         start=(cc == 0), stop=(cc == 7))
                i0 = F * 4
                psv_v = psv[:].rearrange("p (t c) -> p t c", t=4)
                nc.vector.tensor_add(va[0][:, i0:i0 + 4, 0:HS],
                                     psv_v[:, :, 0:HS],
                                     bv_bc[:, 0:HS].unsqueeze(1).broadcast_to([128, 4, HS]))
                nc.vector.tensor_add(va[1][:, i0:i0 + 4, 0:HS],
                                     psv_v[:, :, HS:HS2],
                                     bv_bc[:, HS:HS2].unsqueeze(1).broadcast_to([128, 4, HS]))
                yield

        def gen_attn(b, st):
            qT = st["qT"]
            kT = st["kT"]
            va = st["va"]
            yT = ytpool.tile([HS2, TB], bf16, name="yT")
            st["yT"] = yT
            yt_u = [ytmpp.tile([HS + 1, TB], f32, name=f"ytmp{h}")
                    for h in range(NH_LOC)]
            pTs = [None] * NQB

            def st_steps(qb):
                nch = 2 * qb + 2
                qcols = bass.ds(qb * QB, QB)
                pT = pTs[qb]
                for g in range(0, nch, EXPG):
                    last = g + EXPG >= nch
                    stp = stps.tile([128, 2, EXPG * QB], f32, name="stp", tag="stp")
                    for j in range(g, g + EXPG):
                        # the final k-chunk of the block is fully masked for
                        # the first 128 q columns: compute only the live half
                        off = (j - g) * QB
                        w = KC if (last and j == g + EXPG - 1) else QB
                        qc = bass.ds(qb * QB + (QB - w), w)
                        for h in range(NH_LOC):
                            # row-tiled pair: head h contracts over its own
                            # 64 q/k dims at PE tile position (64*h, 0)
                            nc.tensor.matmul(
                                stp[:, h, off:off + w],
                                kT[h * HS:(h + 1) * HS, j * KC:(j + 1) * KC],
                                qT[h * HS:(h + 1) * HS, qc],
                                start=True, stop=True)
                        yield
                    gw = QB + KC if last else EXPG * QB
                    if last:
                        # additive causal mask on the diagonal k-chunks
                        nc.vector.tensor_add(stp[:, :, 0:gw], stp[:, :, 0:gw],
                                             mn_sb[:])
                    nc.scalar.activation(pT[:, :, g * QB:g * QB + gw],
                                         stp[:, :, 0:gw], Exp, scale=SCALE)
                    yield

            def pv_steps(qb, norm_half=None):
                nch = 2 * qb + 2
                qcols = bass.ds(qb * QB, QB)
                pT = pTs[qb]
                for h in range(NH_LOC):
                    pvp_t = pvps.tile([128, 512], f32, name="pv", tag="pv")
                    pvp = pvp_t[0:HS + 1, 0:QB]
                    for j in range(nch):
                        w = KC if j == nch - 1 else QB
                        nc.tensor.matmul(pvp_t[0:HS + 1, QB - w:QB],
                                         va[h][:, j, :],
                                         pT[:, h, j * QB:j * QB + w],
                                         start=(j == 0), stop=(j == nch - 1))
                        if j % 2 == 1:
                            yield
                    nc.vector.tensor_copy(yt_u[h][:, qcols], pvp)
                    if norm_half is not None:
                        # batch tail: start head h's normalizer chain while
                        # the other head's PV still streams on the PE.  The
                        # last batch only normalizes its final quarter here
                        # (the third quarter ran mid-attention).
                        if b == B - 1:
                            emit_norm(b, h, yt_u[h], yT, TB - TB // 4, TB // 4,
                                      tail=True)
                        else:
                            emit_norm(b, h, yt_u[h], yT, TB // 2, TB // 2,
                                      tail=True)
                    yield

            for qb in range(NQB):
                pTs[qb] = ptp.tile([128, 2, 16 * QB], bf16, name="pT", tag="pT")
                streams = [st_steps(qb)]
                if qb > 0:
                    streams.append(pv_steps(qb - 1))
                alive = list(streams)
                while alive:
                    for g in list(alive):
                        try:
                            next(g)
                        except StopIteration:
                            alive.remove(g)
                if qb == NQB // 2:
                    # q-columns 0:TB/2 are final once pv(NQB/2-1) is done:
                    # normalize the first half early so proj(b) can start
                    # while the second half of attention still runs.
                    for h in range(NH_LOC):
                        emit_norm(b, h, yt_u[h], yT, 0, TB // 2)
                if b == B - 1 and qb == NQB - 2:
                    # last batch: the third quarter is final once pv(NQB-3)
                    # is done — normalize it now so proj units for those
                    # tokens join the drain filler instead of the gated tail
                    for h in range(NH_LOC):
                        emit_norm(b, h, yt_u[h], yT, TB // 2, TB // 4)
                yield
            for _ in pv_steps(NQB - 1, norm_half=1):
                yield

        def emit_norm(b, h, yt_u, yT, lo, width, tail=False):
            # 1/l with l reshaped to [128,H] (a 1-partition reciprocal is
            # ~6.3ns/elem serial on DVE), then partition-broadcast via DRAM.
            HB = TB // 2
            hc = bass.ds(lo, width)
            l128 = lrp.tile([128, HB // 128], f32, name="l128")
            l128v = l128[:, 0:width // 128]
            nc.sync.dma_start(out=l128v, in_=yt_u[HS:HS + 1, hc])
            l128r = lrp.tile([128, HB // 128], f32, name="l128r")
            nc.vector.reciprocal(l128r[:, 0:width // 128], l128v)
            lb = lb_d[h]
            nc.sync.dma_start(out=lb[lo:lo + width], in_=l128r[:, 0:width // 128])
            rec = recp.tile([64, HB], f32, name="rec")
            bc_ap = bass.AP(lb.tensor, lb.offset + lo, [[0, 64], [1, width]])
            nc.sync.dma_start(out=rec[:, 0:width], in_=bc_ap)
            # tail chains: run the two heads' muls on different engines so
            # they overlap; split each mul so gated proj unblocks sooner
            eng = nc.vector if (tail and h == 1) else nc.gpsimd
            QH = min(width, HB // 2)
            for qtr in range(width // QH):
                qc = bass.ds(lo + qtr * QH, QH)
                eng.tensor_mul(yT[h * 64:(h + 1) * 64, qc], yt_u[0:HS, qc],
                               rec[:, qtr * QH:(qtr + 1) * QH])

        def gen_proj(b, st):
            yT = st["yT"]
            base = b * TB
            for i2 in range(TB // 256):
                osb = outp.tile([128, 2, C], bf16, name="osb")
                for sub in range(2):
                    i = i2 * 2 + sub
                    for nb in range(2):
                        # the last batch has no QKV weave, so mmps is free:
                        # alternate pools for a 4-bank rotation so the drain
                        # matmuls pipeline past the evict latency
                        if b == B - 1 and nb == 1:
                            pp = mmps.tile([128, 512], f32, name="mm", tag="mm")
                        else:
                            pp = pvps.tile([128, 512], f32, name="pv", tag="pv")
                        nc.tensor.matmul(pp[:], yT[:, i * 128:(i + 1) * 128],
                                         wp_sb[:, nb * 512:(nb + 1) * 512],
                                         start=True, stop=True)
                        if nb == 0:
                            nc.scalar.copy(osb[:, sub, 0:512], pp[:])
                        else:
                            nc.vector.tensor_copy(osb[:, sub, 512:1024], pp[:])
                # the very last units' DMAs are pure tail: split them so the
                # transfer spreads across queues instead of ~11us on one
                nsplit = 4 if (b == B - 1 and i2 >= TB // 256 - 2) else 1
                rows = 128 // nsplit
                for sub in range(2):
                    i = i2 * 2 + sub
                    for sp in range(nsplit):
                        r0 = base + i * 128 + sp * rows
                        nc.sync.dma_start(y_d[r0:r0 + rows, :],
                                          osb[sp * rows:(sp + 1) * rows, sub, :])
                yield

        # ---- schedule ---------------------------------------------------
        # Batch 0 prologue: weave attn(0) q-blocks behind the qkv(0)
        # F-blocks that produce their inputs (S^T of qb needs q/k cols up to
        # (qb+1)*256; PV lags one qb behind).
        states = {0: {}}
        g0 = gen_qkv(0, states[0])
        a0 = gen_attn(0, states[0])
        nq_needed = [3, 3, 6, 6, 9, 9, 12, 12]
        q_done = 0
        for qb in range(NQB):
            while q_done < nq_needed[qb]:
                next(g0)
                q_done += 1
            next(a0)
        a0_tail = a0

        # Main loop: attn(b) woven with qkv(b+1) (front-loaded so kT/qT of
        # b+1 are ready before attn(b+1) starts), the second half of
        # proj(b-1), and — once the early half-norm has run — the first
        # half of proj(b) itself.
        for b in range(B):
            qkv_g = None
            if b + 1 < B:
                states[b + 1] = {}
                qkv_g = gen_qkv(b + 1, states[b + 1])
            proj_prev = states[b - 1].get("proj_g") if b > 0 else None
            proj_self = gen_proj(b, states[b])
            states[b]["proj_g"] = proj_self
            self_emitted = 0
            attn_g = a0_tail if b == 0 else gen_attn(b, states[b])
            slot = 8 if b == 0 else 0
            last = b == B - 1
            for _ in attn_g:
                slot += 1
                if qkv_g is not None:
                    try:
                        next(qkv_g)
                    except StopIteration:
                        qkv_g = None
                if proj_prev is not None and slot >= 4:
                    # units 4+ of proj(b-1) are gated on the end-of-batch
                    # norm chain of b-1 (~8us of DMA round-trips): weaving
                    # them at slot 1 head-of-line-blocks the in-order PE
                    # queue behind that chain
                    try:
                        next(proj_prev)
                    except StopIteration:
                        proj_prev = None
                if slot >= 7 and self_emitted < (TB // 256) // 2 and not last:
                    next(proj_self)
                    self_emitted += 1
            for g in (qkv_g, proj_prev):
                if g is not None:
                    for _ in g:
                        pass
        # Drain proj(B-1): the first half (gated on the mid-batch norm,
        # long done) fills the PE while the half-1 norm chain completes.
        for _ in states[B - 1]["proj_g"]:
            pass


def _get_nc():
    if "nc" not in _CACHE:
        _CACHE["nc"] = _build()
    return _CACHE["nc"]


def make_in_maps(x, w_attn, b_attn, w_proj, b_proj):
    bf = ml_dtypes.bfloat16
    x = np.asarray(x, dtype=np.float32)
    w_attn = np.asarray(w_attn, dtype=np.float32)
    b_attn = np.asarray(b_attn, dtype=np.float32)
    w_proj = np.asarray(w_proj, dtype=np.float32)

    xTh = np.ascontiguousarray(x.reshape(TOK, C).T.astype(bf))
    r = np.arange(KC)[:, None]
    s = np.arange(QB)[None, :]
    m0 = np.where(r <= s, 0.0, NEG).astype(np.float32)
    mblock = np.concatenate([m0, m0[:, :KC]], axis=1)  # [128, 384]
    mneg = np.tile(mblock, (1, 2))                     # [128, 768] (2 heads)

    in_maps = []
    for c in range(NCORES):
        hc = slice(c * HS2, (c + 1) * HS2)
        in_maps.append({
            "xT": xTh,
            "wq": np.ascontiguousarray(w_attn[:, hc].astype(bf)),
            "wk": np.ascontiguousarray(w_attn[:, C + c * HS2:C + (c + 1) * HS2].astype(bf)),
            "wv": np.ascontiguousarray(w_attn[:, 2 * C + c * HS2:2 * C + (c + 1) * HS2].astype(bf)),
            "wp": np.ascontiguousarray(w_proj[hc, :].astype(bf)),
            "bq": np.ascontiguousarray(b_attn[hc]).reshape(HS2, 1),
            "bk": np.ascontiguousarray(b_attn[C + c * HS2:C + (c + 1) * HS2]).reshape(HS2, 1),
            "bv": np.ascontiguousarray(b_attn[2 * C + c * HS2:2 * C + (c + 1) * HS2]).reshape(HS2, 1),
            "mneg": mneg,
        })
    return in_maps


def kernel(x, w_attn, b_attn, w_proj, b_proj):
    from concourse.bass_utils import run_bass_kernel_spmd

    b_proj = np.asarray(b_proj, dtype=np.float32)
    in_maps = make_in_maps(x, w_attn, b_attn, w_proj, b_proj)
    nc = _get_nc()
    res = run_bass_kernel_spmd(nc, in_maps, core_ids=list(range(NCORES)))
    y = res.results[0]["y"].astype(np.float32).copy()
    for c in range(1, NCORES):
        y += res.results[c]["y"].astype(np.float32)
    y += b_proj[None, :]
    return y.reshape(B, T, C)
